# revision 42
# baseline (speedup 1.0000x reference)
import sys

for _p in ("/opt/trn_rl_repo", "/opt/trn_rl_repo/concourse"):
    if _p not in sys.path:
        sys.path.insert(0, _p)

import numpy as np
import ml_dtypes

N_CORES = 8
B, H, W_DIM, C = 8, 32, 32, 288
N = H * W_DIM          # 1024 points per core (batch-dim sharding: 1 image per core)
O = 64                 # codewords
SLAB = 32              # c-slab height; 288 = 9 slabs, zero padding
NSLAB = C // SLAB      # 9
GRP = 4                # o's packed per 128-partition tile (4 x 32)
NGRP = O // GRP        # 16
CHUNK = 512            # PSUM bank free size (fp32)
NCH = N // CHUNK       # 2 chunks
PTBUF = 18             # DVE p-tile ring depth (2+ full groups of 7)
SC_SLABS = (3, 7)      # slabs per group computed on the Scalar engine
SCBUF = 6              # scalar p-tile ring depth (3 groups of 2)
CP3 = 384              # C padded to 3 full 128-partition tiles (for -Sx)

_CACHE = {}
_DEBUG_NAMES = {}


def _patch_drain_split():
    # The end-of-TileContext drain waits on the FULL global clock (PE + DVE
    # + one sem per DMA HW queue), overflowing the CTRL_NO struct's
    # sync-wait slots in walrus. Split: emit one 1-wait SP nop per clock
    # component first; the original drain's full-clock add_sem_waits then
    # elides everything via SP wait history.
    import concourse.tile as tile_mod
    from concourse.vector_clock import ScopedClock, VectorClock

    if getattr(tile_mod.TileContext, "_drain_split_patched", False):
        return

    def _drain_and_barrier(self, tick_clock, wait_clock):
        gc = tick_clock.global_clock
        for idx in range(len(gc)):
            tick = gc[idx]
            if tick <= 0:
                continue
            nop = self.nc.sync.nop(nofuse=True, hint="drain_split")
            vc = VectorClock()
            vc.require_at_least(idx, tick)
            wait_clock.add_sem_waits(nop.ins, ScopedClock({None: vc}))
        # Waitless drain: the nops above (same SP sequencer, in order)
        # already guarantee every sem is at its final value here.
        self.nc.sync.drain()
        self.nc.all_engine_barrier()
        assert self.sems is not None
        popped = self.nc._tile_sem_poison_stack.pop()
        assert popped is self._sem_poison
        self.nc.clear_and_free_semaphores(list(self.sems.allocated().values()))
        self.nc.all_engine_barrier()

    tile_mod.TileContext._drain_and_barrier = _drain_and_barrier
    tile_mod.TileContext._drain_split_patched = True


def _dedup_ldweights(nc):
    # 18 consecutive matmuls per group share one lhsT window, but the tile
    # flow emits an explicit Ldweights per matmul (~100ns each on the PE
    # pipe). PE weight state is sticky, so a Ldweights identical to the
    # previous one on the (in-order) PE stream is a no-op: delete it.
    # Keep any that carry sync waits/updates.
    for f in nc.m.functions:
        for bb in f.blocks:
            il = bb.instructions
            last_sig = None
            kill = []
            for idx in range(len(il)):
                i = il[idx]
                if i.opcode != "Ldweights":
                    continue
                sig = (
                    str(i.ins[0]),
                    str(getattr(i, "perf_mode", None)),
                    str(getattr(i, "tile_position", None)),
                    str(getattr(i, "tile_size", None)),
                    str(getattr(i, "is_transpose", None)),
                )
                has_sync = bool(
                    i.sync_info
                    and (i.sync_info.on_wait or i.sync_info.on_update)
                )
                if sig == last_sig and not has_sync:
                    kill.append(idx)
                else:
                    last_sig = sig
            for idx in reversed(kill):
                del il[idx]


def _strip_out_dma_queue_wait(nc):
    # The final out-DMA carries {DVE data wait, DMAHW queue-ring wait}; the
    # HWDGE descriptor encodes only one. The extraction the DVE wait points
    # at transitively depends on every input DMA (psum <- matmuls <- TS/ACT
    # <- all input tiles), so the queue-ring wait is redundant: drop it.
    for f in nc.m.functions:
        for bb in f.blocks:
            il = bb.instructions
            for idx in range(len(il) - 1, -1, -1):
                i = il[idx]
                if i.opcode != "DMACopy":
                    continue
                si = i.sync_info
                if not si or not si.on_wait or len(si.on_wait) < 2:
                    return
                keeps = [w for w in si.on_wait if "DMAHW" not in str(w)]
                drops = [w for w in si.on_wait if "DMAHW" in str(w)]
                assert len(keeps) == 1 and len(drops) >= 1, (keeps, drops)
                while si.on_wait:
                    si.on_wait.pop()
                si.on_wait.extend(keeps)
                return


def _build_program():
    import concourse.bass as bass
    import concourse.tile as tile
    from concourse import mybir

    _patch_drain_split()
    nc = bass.Bass("TRN2", debug=False, num_devices=N_CORES)

    # xrep: slab s (32 c's) replicated 4x across the partition dim, bf16.
    xrep_d = nc.dram_tensor("xrep", [NSLAB * 128, N], mybir.dt.bfloat16, kind="ExternalInput")
    # xt3: plain transposed x, zero-padded to 384 c's (for the -Sx matmuls).
    xt3_d = nc.dram_tensor("xt3", [CP3, N], mybir.dt.bfloat16, kind="ExternalInput")
    # wneg: column 9*g+s = -w packed per (group, slab): [32k+i] = -w[32s+i, 4g+k]
    wneg_d = nc.dram_tensor("wneg", [128, NGRP * NSLAB], mybir.dt.float32, kind="ExternalInput")
    # b2: [p, j] = (b[j] + sum_c w[c,j]) / 128, bf16 (bias via rank-1 matmul)
    b2_d = nc.dram_tensor("b2", [128, O], mybir.dt.bfloat16, kind="ExternalInput")
    out_d = nc.dram_tensor("out_t", [O, N], mybir.dt.float32, kind="ExternalOutput")

    xrep = xrep_d.ap()
    xt3 = xt3_d.ap()
    wneg = wneg_d.ap()
    b2 = b2_d.ap()
    out_t = out_d.ap()

    from contextlib import ExitStack

    from concourse.tile import add_dep_helper

    with tile.TileContext(nc) as tc, ExitStack() as ctx:
        const_pool = ctx.enter_context(tc.tile_pool(name="const", bufs=1))
        psum_pool = ctx.enter_context(tc.tile_pool(name="ps", bufs=1, space="PSUM"))

        # Walrus TensorScalar/Activation ISA structs fit ONE sync wait.
        # Every DMA gets a tiny DVE "touch" so later DVE consumers carry the
        # DMA-queue wait in DVE history; all DVE-sourced deps merge into the
        # single per-engine sem component.
        scratch = const_pool.tile([1, 128], mybir.dt.float32)
        touch_col = [0]

        def touch(src_ap):
            k = touch_col[0]
            touch_col[0] += 1
            bi = nc.vector.tensor_scalar_add(scratch[0:1, k : k + 1], src_ap, 0.0)
            return bi, k

        def touch_write(tile_obj):
            # write into a ring slot, reading only the long-quiet scratch
            # col 127 so the sole fresh wait is the slot's PE reader clock.
            # The write straddles the chunk boundary so it WARs against
            # BOTH chunk matmuls (subtile deps track per-range readers).
            bi = nc.vector.tensor_scalar_add(
                tile_obj[0:1, CHUNK - 1 : CHUNK + 1], scratch[0:1, 126:128], 0.0
            )
            return bi, None

        nc.vector.memset(scratch[:], 0.0)
        # cache-key marker: BIR must differ from the ldw-opt=false build
        # (walrus flags are not part of the NEFF cache key)
        nc.vector.memset(scratch[0:1, 125:126], 1.0)

        # Scalar-engine mirror of the touch machinery (its own scratch and
        # wait history). All scalar ops use Relu so the activation table
        # loads once.
        sscratch = const_pool.tile([1, 128], mybir.dt.float32)
        stouch_col = [0]

        def stouch(src_ap):
            k = stouch_col[0]
            stouch_col[0] += 1
            bi = nc.scalar.activation(
                sscratch[0:1, k : k + 1], src_ap,
                mybir.ActivationFunctionType.Relu, bias=0.0, scale=1.0,
            )
            return bi, k

        def stouch_write(tile_obj):
            bi = nc.scalar.activation(
                tile_obj[0:1, CHUNK - 1 : CHUNK + 1], sscratch[0:1, 126:128],
                mybir.ActivationFunctionType.Relu, bias=0.0, scale=1.0,
            )
            return bi, None

        in_dmas = []

        # Issue order follows first-need: x0 + wneg feed the first relu
        # tile; later slabs stream in at the pace the producers consume
        # them; xt3/b2 (the closing -Sx/bias matmuls) come last. Split
        # across both HWDGE engines — each issue costs ~0.6us on its
        # issuing engine and SP alone serializes the whole set.
        x_sb = [
            const_pool.tile([128, N], mybir.dt.bfloat16, name=f"x_sb{s}")
            for s in range(NSLAB)
        ]

        def dma_x(s, eng):
            in_dmas.append(
                eng.dma_start(x_sb[s][:], xrep[128 * s : 128 * (s + 1), :])
            )
            touch(x_sb[s][0:1, 0:1])
            if s in SC_SLABS:
                stouch(x_sb[s][0:1, 0:1])

        dma_x(0, nc.sync)

        wneg_sb = const_pool.tile([128, NGRP * NSLAB], mybir.dt.float32)
        in_dmas.append(nc.scalar.dma_start(wneg_sb[:], wneg[:, :]))
        touch(wneg_sb[0:1, 0:1])
        # scalar history preload for wneg; also initializes sscratch cols
        # 120..127 (stouch_write reads 126:128)
        nc.scalar.activation(
            sscratch[0:1, 120:128], wneg_sb[0:1, 0:8],
            mybir.ActivationFunctionType.Relu, bias=0.0, scale=1.0,
        )

        for s in range(1, NSLAB):
            dma_x(s, nc.sync if s % 2 == 0 else nc.scalar)

        xt3_sb = []
        for t in range(3):
            xs = const_pool.tile([128, N], mybir.dt.bfloat16, name=f"xt3_{t}")
            eng = nc.sync if t % 2 == 0 else nc.scalar
            in_dmas.append(eng.dma_start(xs[:], xt3[128 * t : 128 * (t + 1), :]))
            touch(xs[0:1, 0:1])
            xt3_sb.append(xs)

        b2_sb = const_pool.tile([128, O], mybir.dt.bfloat16)
        in_dmas.append(nc.scalar.dma_start(b2_sb[:], b2[:, :]))
        touch(b2_sb[0:1, 0:1])

        # zwin[p, 64 + p//32] = 2.0 else 0. lhsT for group g = zwin[:, 64-4g :
        # 128-4g]: window column j holds the 2.0-block for output partition j
        # exactly when j = 4g + p//32 — routes 2*sum_c(relu) of o-block k
        # onto PSUM partition 4g+k.
        zwin = const_pool.tile([128, 128], mybir.dt.bfloat16)
        nc.vector.memset(zwin[:], 0.0)
        for k in range(GRP):
            nc.vector.memset(zwin[32 * k : 32 * (k + 1), 64 + k : 65 + k], 2.0)

        # all-(-1) lhsT: -Sx[n] accumulated onto every output partition
        neg1 = const_pool.tile([128, O], mybir.dt.bfloat16)
        nc.vector.memset(neg1[:], -1.0)
        # all-ones rhs for the rank-1 bias matmul
        ones = const_pool.tile([128, CHUNK], mybir.dt.bfloat16)
        nc.vector.memset(ones[:], 1.0)

        ps = [
            psum_pool.tile([O, CHUNK], mybir.dt.float32, name=f"ps{ch}")
            for ch in range(NCH)
        ]

        # p-tile rings: 18 fixed DVE tiles (7 per group -> ~2.5 groups deep)
        # and 6 scalar tiles (2 per group -> 3 groups deep). A writer
        # rewriting slot j-RING carries a WAR against that slot's old PE
        # readers and a WAW against its old writer; both are pre-absorbed
        # into the writing engine's wait history once per group (rt/at/wt)
        # so each real producer op carries <=1 sync wait (walrus TS/ACT
        # ISA structs fit only one).
        DVE_PER_GRP = NSLAB - len(SC_SLABS)  # 7
        pt = [
            const_pool.tile([128, N], mybir.dt.bfloat16, name=f"pt{j}")
            for j in range(PTBUF)
        ]
        spt = [
            const_pool.tile([128, N], mybir.dt.bfloat16, name=f"spt{j}")
            for j in range(SCBUF)
        ]
        dve_hist = []  # dve tile index -> ring tile
        sc_hist = []

        # out[o, n] = 2*sum_c relu(x-w) - Sx[n] + Sw[o] + b[o]
        #
        # PE warmup: short garbage matmuls (zwin x zwin into a scrap bank)
        # as soon as the memsets land — the p-state ramp (~3us of continuous
        # busy to reach full clock) burns while the x slabs stream in.
        scrap = psum_pool.tile([64, CHUNK], mybir.dt.float32, name="scrap")
        for _ in range(24):
            nc.tensor.matmul(
                scrap[0:64, 0:64],
                lhsT=zwin[:, 64:128],
                rhs=zwin[:, 0:64],
                start=True,
                stop=True,
            )

        for g in range(NGRP):
            # groups 0-1 run all-DVE: the scalar engine's x slabs are still
            # in flight then, and a late scalar tile stalls the whole PE
            # stream at spin-up.
            sc_slabs = SC_SLABS if g >= 2 else ()
            n_dve = NSLAB - len(sc_slabs)
            wt = None
            swt = None
            jt = len(dve_hist) + n_dve - 1 - PTBUF
            if jt >= 0:
                # newest ring slot group g will overwrite: its old writer /
                # PE readers dominate every other slot the group touches.
                rt, krt = touch(dve_hist[jt][0:1, 0:1])
                at, _ = touch(scratch[0:1, krt : krt + 1])
                wt, _ = touch_write(dve_hist[jt])
                _DEBUG_NAMES[rt.ins.name] = f"rt{g}"
                _DEBUG_NAMES[at.ins.name] = f"at{g}"
                _DEBUG_NAMES[wt.ins.name] = f"wt{g}"
            sjt = len(sc_hist) + len(sc_slabs) - 1 - SCBUF
            if sjt >= 0:
                srt, skrt = stouch(sc_hist[sjt][0:1, 0:1])
                sat, _ = stouch(sscratch[0:1, skrt : skrt + 1])
                swt, _ = stouch_write(sc_hist[sjt])
                _DEBUG_NAMES[srt.ins.name] = f"srt{g}"
                _DEBUG_NAMES[sat.ins.name] = f"sat{g}"
                _DEBUG_NAMES[swt.ins.name] = f"swt{g}"

            for s in range(NSLAB):
                i = NSLAB * g + s
                col = i
                if s in sc_slabs:
                    p = spt[len(sc_hist) % SCBUF]
                    sc_hist.append(p)
                    pr = nc.scalar.activation(
                        p[:], x_sb[s][:],
                        mybir.ActivationFunctionType.Relu,
                        bias=wneg_sb[:, col : col + 1], scale=1.0,
                    )
                    gate = swt
                else:
                    p = pt[len(dve_hist) % PTBUF]
                    dve_hist.append(p)
                    pr = nc.vector.tensor_scalar(
                        p[:], x_sb[s][:], wneg_sb[:, col : col + 1], 0.0,
                        op0=mybir.AluOpType.add,
                        op1=mybir.AluOpType.max,
                    )
                    gate = wt
                _DEBUG_NAMES[pr.ins.name] = f"p{i}"
                if gate is not None:
                    # scheduling-only edge: keep every producer of this
                    # group after the group's absorber, so the PE wait is
                    # already in the engine's history when it is placed.
                    add_dep_helper(pr.ins, gate.ins, sync=False,
                                   reason="producer after group absorber")
                for ch in range(NCH):
                    nc.tensor.matmul(
                        ps[ch][:],
                        lhsT=zwin[:, 64 - 4 * g : 128 - 4 * g],
                        rhs=p[:, CHUNK * ch : CHUNK * (ch + 1)],
                        start=(g == 0 and s == 0),
                        stop=False,
                    )

        # -Sx[n]: sum over all c (3 padded 128-tiles) with weight -1
        for t in range(3):
            for ch in range(NCH):
                nc.tensor.matmul(
                    ps[ch][:],
                    lhsT=neg1[:, 0:O],
                    rhs=xt3_sb[t][:, CHUNK * ch : CHUNK * (ch + 1)],
                    start=False,
                    stop=False,
                )
        # + (b[o] + Sw[o]): rank-1 matmul, lhsT column j = (b[j]+Sw[j])/128
        for ch in range(NCH):
            nc.tensor.matmul(
                ps[ch][:],
                lhsT=b2_sb[:, 0:O],
                rhs=ones[:, :],
                start=False,
                stop=True,
            )

        out_sb = const_pool.tile([O, N], mybir.dt.float32)
        for ch in range(NCH):
            nc.vector.tensor_scalar_add(
                out_sb[:, CHUNK * ch : CHUNK * (ch + 1)], ps[ch][:], 0.0
            )
        # SP out-DMA; _strip_out_dma_queue_wait removes the redundant
        # DMA-queue ring wait walrus can't encode alongside the data wait.
        nc.sync.dma_start(out_t[:, :], out_sb[:])

    _dedup_ldweights(nc)
    _strip_out_dma_queue_wait(nc)
    return nc


def _prep_inputs(x, w, b):
    xs = x.reshape(B, N, C).astype(np.float32)
    wf = w.astype(np.float32)

    # wneg[32k+i, 9g+s] = -w[32s+i, 4g+k]
    wneg = np.empty((128, NGRP * NSLAB), dtype=np.float32)
    for g in range(NGRP):
        for s in range(NSLAB):
            blk = -wf[SLAB * s : SLAB * (s + 1), GRP * g : GRP * (g + 1)]  # [32, 4]
            wneg[:, NSLAB * g + s] = blk.T.reshape(128)

    sw = wf.sum(axis=0, dtype=np.float64)
    b2row = ((b.astype(np.float64) + sw) / 128.0).astype(ml_dtypes.bfloat16)
    b2 = np.broadcast_to(b2row, (128, O)).copy()

    in_maps = []
    for core in range(N_CORES):
        xt = xs[core].T.astype(ml_dtypes.bfloat16)  # [288, 1024]
        xrep = np.empty((NSLAB * 128, N), dtype=ml_dtypes.bfloat16)
        for s in range(NSLAB):
            slab = xt[SLAB * s : SLAB * (s + 1), :]  # [32, 1024]
            xrep[128 * s : 128 * (s + 1), :] = np.tile(slab, (GRP, 1))
        xt3 = np.zeros((CP3, N), dtype=ml_dtypes.bfloat16)
        xt3[:C, :] = xt
        in_maps.append({"xrep": xrep, "xt3": xt3, "wneg": wneg, "b2": b2})
    return in_maps


def kernel(x, w, b):
    from concourse.bass_utils import run_bass_kernel_spmd

    if "nc" not in _CACHE:
        _CACHE["nc"] = _build_program()
    nc = _CACHE["nc"]

    in_maps = _prep_inputs(x, w, b)
    res = run_bass_kernel_spmd(nc, in_maps, list(range(N_CORES)))
    out = np.stack(
        [np.asarray(res.results[core]["out_t"], dtype=np.float32).T for core in range(N_CORES)]
    )
    return out.astype(np.float32)


# revision 43
# speedup vs baseline: 1.0761x; 1.0761x over previous
import sys

for _p in ("/opt/trn_rl_repo", "/opt/trn_rl_repo/concourse"):
    if _p not in sys.path:
        sys.path.insert(0, _p)

import numpy as np
import ml_dtypes

N_CORES = 8
B, H, W_DIM, C = 8, 32, 32, 288
N = H * W_DIM          # 1024 points per core (batch-dim sharding: 1 image per core)
O = 64                 # codewords
CHUNK = 512            # PSUM bank free size (fp32)
NCH = N // CHUNK       # 2 chunks
NGRP = 16              # groups of 4 o's (for the 32-c remainder packing)
PTBUF = 16             # DVE p-tile ring (2 groups of 8 full tiles)
SCBUF = 4              # scalar remainder-tile ring (4 groups deep)
CP3 = 384              # C padded to 3 full 128-partition tiles

_CACHE = {}
_DEBUG_NAMES = {}


def _patch_drain_split():
    # The end-of-TileContext drain waits on the FULL global clock (PE + DVE
    # + one sem per DMA HW queue), overflowing the CTRL_NO struct's
    # sync-wait slots in walrus. Split: emit one 1-wait SP nop per clock
    # component first; the original drain's full-clock add_sem_waits then
    # elides everything via SP wait history.
    import concourse.tile as tile_mod
    from concourse.vector_clock import ScopedClock, VectorClock

    if getattr(tile_mod.TileContext, "_drain_split_patched", False):
        return

    def _drain_and_barrier(self, tick_clock, wait_clock):
        gc = tick_clock.global_clock
        for idx in range(len(gc)):
            tick = gc[idx]
            if tick <= 0:
                continue
            nop = self.nc.sync.nop(nofuse=True, hint="drain_split")
            vc = VectorClock()
            vc.require_at_least(idx, tick)
            wait_clock.add_sem_waits(nop.ins, ScopedClock({None: vc}))
        # Waitless drain: the nops above (same SP sequencer, in order)
        # already guarantee every sem is at its final value here.
        self.nc.sync.drain()
        self.nc.all_engine_barrier()
        assert self.sems is not None
        popped = self.nc._tile_sem_poison_stack.pop()
        assert popped is self._sem_poison
        self.nc.clear_and_free_semaphores(list(self.sems.allocated().values()))
        self.nc.all_engine_barrier()

    tile_mod.TileContext._drain_and_barrier = _drain_and_barrier
    tile_mod.TileContext._drain_split_patched = True


def _dedup_ldweights(nc):
    # Consecutive matmuls often share one lhsT window, but the tile flow
    # emits an explicit Ldweights per matmul (~100ns each on the PE pipe).
    # PE weight state is sticky, so a Ldweights identical to the previous
    # one on the (in-order) PE stream is a no-op: delete it. Keep any that
    # carry sync waits/updates.
    for f in nc.m.functions:
        for bb in f.blocks:
            il = bb.instructions
            last_sig = None
            kill = []
            for idx in range(len(il)):
                i = il[idx]
                if i.opcode != "Ldweights":
                    continue
                sig = (
                    str(i.ins[0]),
                    str(getattr(i, "perf_mode", None)),
                    str(getattr(i, "tile_position", None)),
                    str(getattr(i, "tile_size", None)),
                    str(getattr(i, "is_transpose", None)),
                )
                has_sync = bool(
                    i.sync_info
                    and (i.sync_info.on_wait or i.sync_info.on_update)
                )
                if sig == last_sig and not has_sync:
                    kill.append(idx)
                else:
                    last_sig = sig
            for idx in reversed(kill):
                del il[idx]


def _strip_out_dma_queue_wait(nc):
    # The final out-DMA carries {DVE data wait, DMAHW queue-ring wait}; the
    # HWDGE descriptor encodes only one. The extraction the DVE wait points
    # at transitively depends on every input DMA (psum <- matmuls <- TS/ACT
    # <- all input tiles), so the queue-ring wait is redundant: drop it.
    for f in nc.m.functions:
        for bb in f.blocks:
            il = bb.instructions
            for idx in range(len(il) - 1, -1, -1):
                i = il[idx]
                if i.opcode != "DMACopy":
                    continue
                si = i.sync_info
                if not si or not si.on_wait or len(si.on_wait) < 2:
                    return
                keeps = [w for w in si.on_wait if "DMAHW" not in str(w)]
                drops = [w for w in si.on_wait if "DMAHW" in str(w)]
                assert len(keeps) == 1 and len(drops) >= 1, (keeps, drops)
                while si.on_wait:
                    si.on_wait.pop()
                si.on_wait.extend(keeps)
                return


def _build_program():
    import concourse.bass as bass
    import concourse.tile as tile
    from concourse import mybir

    _patch_drain_split()
    nc = bass.Bass("TRN2", debug=False, num_devices=N_CORES)

    # xt3: transposed x, zero-padded to 384 c's. Feeds the full-c-tile relu
    # producers AND the -Sx matmuls (no replication needed for c 0..255).
    xt3_d = nc.dram_tensor("xt3", [CP3, N], mybir.dt.bfloat16, kind="ExternalInput")
    # xrep8: the 32-c remainder (c 256..287) replicated 4x across the
    # partition dim -- one tile shared by every 4-o group.
    xrep8_d = nc.dram_tensor("xrep8", [128, N], mybir.dt.bfloat16, kind="ExternalInput")
    # wnegf: col 2o+t = -w[128t : 128t+128, o] (full tiles)
    wnegf_d = nc.dram_tensor("wnegf", [128, 2 * O], mybir.dt.float32, kind="ExternalInput")
    # wnegr: col g, row 32k+i = -w[256+i, 4g+k] (remainder packing)
    wnegr_d = nc.dram_tensor("wnegr", [128, NGRP], mybir.dt.float32, kind="ExternalInput")
    # b2: [p, j] = (b[j] + sum_c w[c,j]) / 128, bf16 (bias via rank-1 matmul)
    b2_d = nc.dram_tensor("b2", [128, O], mybir.dt.bfloat16, kind="ExternalInput")
    out_d = nc.dram_tensor("out_t", [O, N], mybir.dt.float32, kind="ExternalOutput")

    xt3 = xt3_d.ap()
    xrep8 = xrep8_d.ap()
    wnegf = wnegf_d.ap()
    wnegr = wnegr_d.ap()
    b2 = b2_d.ap()
    out_t = out_d.ap()

    from contextlib import ExitStack
    from concourse.tile import add_dep_helper

    with tile.TileContext(nc) as tc, ExitStack() as ctx:
        const_pool = ctx.enter_context(tc.tile_pool(name="const", bufs=1))
        psum_pool = ctx.enter_context(tc.tile_pool(name="ps", bufs=1, space="PSUM"))

        # Walrus TensorScalar/Activation ISA structs fit ONE sync wait.
        # Every DMA gets a tiny DVE "touch" so later DVE consumers carry the
        # DMA-queue wait in DVE history; all DVE-sourced deps merge into the
        # single per-engine sem component. The scalar engine has its own
        # mirror (sscratch/stouch), all Relu so the act table loads once.
        scratch = const_pool.tile([1, 128], mybir.dt.float32)
        touch_col = [0]

        def touch(src_ap):
            k = touch_col[0]
            touch_col[0] += 1
            bi = nc.vector.tensor_scalar_add(scratch[0:1, k : k + 1], src_ap, 0.0)
            return bi, k

        def touch_write(tile_obj):
            # write into a ring slot, reading only the long-quiet scratch
            # col 127 so the sole fresh wait is the slot's PE reader clock.
            # The write straddles the chunk boundary so it WARs against
            # BOTH chunk matmuls (subtile deps track per-range readers).
            bi = nc.vector.tensor_scalar_add(
                tile_obj[0:1, CHUNK - 1 : CHUNK + 1], scratch[0:1, 126:128], 0.0
            )
            return bi, None

        nc.vector.memset(scratch[:], 0.0)

        sscratch = const_pool.tile([1, 128], mybir.dt.float32)
        stouch_col = [0]

        def stouch(src_ap):
            k = stouch_col[0]
            stouch_col[0] += 1
            bi = nc.scalar.activation(
                sscratch[0:1, k : k + 1], src_ap,
                mybir.ActivationFunctionType.Relu, bias=0.0, scale=1.0,
            )
            return bi, k

        def stouch_write(tile_obj):
            bi = nc.scalar.activation(
                tile_obj[0:1, CHUNK - 1 : CHUNK + 1], sscratch[0:1, 126:128],
                mybir.ActivationFunctionType.Relu, bias=0.0, scale=1.0,
            )
            return bi, None

        in_dmas = []

        # first-need order; split issues across both HWDGE engines (each
        # costs ~0.6us on its issuing engine)
        xt3_sb = []
        for t in range(3):
            xs = const_pool.tile([128, N], mybir.dt.bfloat16, name=f"xt3_{t}")
            eng = nc.sync if t != 1 else nc.scalar
            in_dmas.append(eng.dma_start(xs[:], xt3[128 * t : 128 * (t + 1), :]))
            touch(xs[0:1, 0:1])
            if t < 2:
                stouch(xs[0:1, 0:1])
            xt3_sb.append(xs)

        wnegf_sb = const_pool.tile([128, 2 * O], mybir.dt.float32)
        in_dmas.append(nc.scalar.dma_start(wnegf_sb[:], wnegf[:, :]))
        touch(wnegf_sb[0:1, 0:1])
        # scalar history preload; also initializes sscratch cols 120..127
        # (stouch_write reads 126:128)
        nc.scalar.activation(
            sscratch[0:1, 120:128], wnegf_sb[0:1, 0:8],
            mybir.ActivationFunctionType.Relu, bias=0.0, scale=1.0,
        )

        xrep8_sb = const_pool.tile([128, N], mybir.dt.bfloat16)
        in_dmas.append(nc.sync.dma_start(xrep8_sb[:], xrep8[:, :]))
        touch(xrep8_sb[0:1, 0:1])
        stouch(xrep8_sb[0:1, 0:1])

        wnegr_sb = const_pool.tile([128, NGRP], mybir.dt.float32)
        in_dmas.append(nc.scalar.dma_start(wnegr_sb[:], wnegr[:, :]))
        touch(wnegr_sb[0:1, 0:1])
        stouch(wnegr_sb[0:1, 0:1])

        b2_sb = const_pool.tile([128, O], mybir.dt.bfloat16)
        in_dmas.append(nc.sync.dma_start(b2_sb[:], b2[:, :]))
        touch(b2_sb[0:1, 0:1])

        # zwin1[p, 64] = 2.0: one-hot window; lhsT for o = zwin1[:, 64-o :
        # 128-o] routes 2*sum_c(rhs) onto PSUM partition o.
        zwin1 = const_pool.tile([128, 128], mybir.dt.bfloat16)
        nc.vector.memset(zwin1[:], 0.0)
        nc.vector.memset(zwin1[:, 64:65], 2.0)
        # zwin2[p, 64 + p//32] = 2.0: block window; lhsT for group g =
        # zwin2[:, 64-4g : 128-4g] routes o-block k onto partition 4g+k.
        zwin2 = const_pool.tile([128, 128], mybir.dt.bfloat16)
        nc.vector.memset(zwin2[:], 0.0)
        for k in range(4):
            nc.vector.memset(zwin2[32 * k : 32 * (k + 1), 64 + k : 65 + k], 2.0)

        ones = const_pool.tile([128, CHUNK], mybir.dt.bfloat16)
        nc.vector.memset(ones[:], 1.0)
        # all-(-1) lhsT: -Sx[n] accumulated onto every output partition
        neg1 = const_pool.tile([128, O], mybir.dt.bfloat16)
        nc.vector.memset(neg1[:], -1.0)

        ps = [
            psum_pool.tile([O, CHUNK], mybir.dt.float32, name=f"ps{ch}")
            for ch in range(NCH)
        ]

        # out[o, n] = 2*sum_c relu(x-w) - Sx[n] + Sw[o] + b[o]
        #
        # -Sx and bias first: xt3 is the first DMA to land, so the PE gets
        # real work (and its p-state ramp) while wnegf/xrep8 stream in.
        for t in range(3):
            for ch in range(NCH):
                nc.tensor.matmul(
                    ps[ch][:],
                    lhsT=neg1[:, 0:O],
                    rhs=xt3_sb[t][:, CHUNK * ch : CHUNK * (ch + 1)],
                    start=(t == 0),
                    stop=False,
                )
        for ch in range(NCH):
            nc.tensor.matmul(
                ps[ch][:],
                lhsT=b2_sb[:, 0:O],
                rhs=ones[:, :],
                start=False,
                stop=False,
            )

        # p-tile rings. A writer rewriting slot j-RING carries a WAR against
        # that slot's old PE readers and a WAW against its old writer; both
        # are pre-absorbed into the writing engine's wait history once per
        # group (rt/at/wt) so each real producer op carries <=1 sync wait.
        pt = [
            const_pool.tile([128, N], mybir.dt.bfloat16, name=f"pt{j}")
            for j in range(PTBUF)
        ]
        spt = [
            const_pool.tile([128, N], mybir.dt.bfloat16, name=f"spt{j}")
            for j in range(SCBUF)
        ]
        dve_hist = []
        sc_hist = []
        sc_prev = None  # remainder tile of the previous group (1-group lead)

        for g in range(NGRP):
            wt = None
            swt = None
            jt = len(dve_hist) + 8 - 1 - PTBUF
            if jt >= 0:
                rt, krt = touch(dve_hist[jt][0:1, 0:1])
                at, _ = touch(scratch[0:1, krt : krt + 1])
                wt, _ = touch_write(dve_hist[jt])
                _DEBUG_NAMES[rt.ins.name] = f"rt{g}"
                _DEBUG_NAMES[at.ins.name] = f"at{g}"
                _DEBUG_NAMES[wt.ins.name] = f"wt{g}"
            sjt = len(sc_hist) - SCBUF
            if sjt >= 0:
                srt, skrt = stouch(sc_hist[sjt][0:1, 0:1])
                sat, _ = stouch(sscratch[0:1, skrt : skrt + 1])
                swt, _ = stouch_write(sc_hist[sjt])
                _DEBUG_NAMES[srt.ins.name] = f"srt{g}"
                _DEBUG_NAMES[sat.ins.name] = f"sat{g}"
                _DEBUG_NAMES[swt.ins.name] = f"swt{g}"

            # remainder tile for THIS group on the scalar engine (consumed
            # at the start of group g+1 -> one group of lead time)
            sp = spt[len(sc_hist) % SCBUF]
            sc_hist.append(sp)
            act = nc.scalar.activation(
                sp[:], xrep8_sb[:],
                mybir.ActivationFunctionType.Relu,
                bias=wnegr_sb[:, g : g + 1], scale=1.0,
            )
            _DEBUG_NAMES[act.ins.name] = f"act{g}"
            if swt is not None:
                add_dep_helper(act.ins, swt.ins, sync=False,
                               reason="producer after group absorber")
            if sc_prev is not None:
                gp = g - 1
                for ch in range(NCH):
                    nc.tensor.matmul(
                        ps[ch][:],
                        lhsT=zwin2[:, 64 - 4 * gp : 128 - 4 * gp],
                        rhs=sc_prev[:, CHUNK * ch : CHUNK * (ch + 1)],
                        start=False,
                        stop=False,
                    )
            sc_prev = sp

            for o in range(4 * g, 4 * g + 4):
                tsl = []
                for t in range(2):
                    p = pt[len(dve_hist) % PTBUF]
                    dve_hist.append(p)
                    ts = nc.vector.tensor_scalar(
                        p[:], xt3_sb[t][:], wnegf_sb[:, 2 * o + t : 2 * o + t + 1],
                        0.0,
                        op0=mybir.AluOpType.add,
                        op1=mybir.AluOpType.max,
                    )
                    _DEBUG_NAMES[ts.ins.name] = f"ts{o}_{t}"
                    if wt is not None:
                        add_dep_helper(ts.ins, wt.ins, sync=False,
                                       reason="producer after group absorber")
                    tsl.append(p)
                for t in range(2):
                    for ch in range(NCH):
                        nc.tensor.matmul(
                            ps[ch][:],
                            lhsT=zwin1[:, 64 - o : 128 - o],
                            rhs=tsl[t][:, CHUNK * ch : CHUNK * (ch + 1)],
                            start=False,
                            stop=False,
                        )

        # last group's remainder closes both accumulations
        gp = NGRP - 1
        for ch in range(NCH):
            nc.tensor.matmul(
                ps[ch][:],
                lhsT=zwin2[:, 64 - 4 * gp : 128 - 4 * gp],
                rhs=sc_prev[:, CHUNK * ch : CHUNK * (ch + 1)],
                start=False,
                stop=True,
            )

        out_sb = const_pool.tile([O, N], mybir.dt.float32)
        for ch in range(NCH):
            nc.vector.tensor_scalar_add(
                out_sb[:, CHUNK * ch : CHUNK * (ch + 1)], ps[ch][:], 0.0
            )
        # SP out-DMA; _strip_out_dma_queue_wait removes the redundant
        # DMA-queue ring wait walrus can't encode alongside the data wait.
        nc.sync.dma_start(out_t[:, :], out_sb[:])

    _dedup_ldweights(nc)
    _strip_out_dma_queue_wait(nc)
    return nc


def _prep_inputs(x, w, b):
    xs = x.reshape(B, N, C).astype(np.float32)
    wf = w.astype(np.float32)

    wnegf = np.empty((128, 2 * O), dtype=np.float32)
    for o in range(O):
        for t in range(2):
            wnegf[:, 2 * o + t] = -wf[128 * t : 128 * (t + 1), o]
    # wnegr[32k+i, g] = -w[256+i, 4g+k]
    wnegr = np.empty((128, NGRP), dtype=np.float32)
    for g in range(NGRP):
        blk = -wf[256:288, 4 * g : 4 * (g + 1)]  # [32, 4]
        wnegr[:, g] = blk.T.reshape(128)

    sw = wf.sum(axis=0, dtype=np.float64)
    b2row = ((b.astype(np.float64) + sw) / 128.0).astype(ml_dtypes.bfloat16)
    b2 = np.broadcast_to(b2row, (128, O)).copy()

    in_maps = []
    for core in range(N_CORES):
        xt = xs[core].T.astype(ml_dtypes.bfloat16)  # [288, 1024]
        xt3 = np.zeros((CP3, N), dtype=ml_dtypes.bfloat16)
        xt3[:C, :] = xt
        xrep8 = np.tile(xt[256:288, :], (4, 1))  # [128, 1024]
        in_maps.append(
            {"xt3": xt3, "xrep8": xrep8, "wnegf": wnegf, "wnegr": wnegr, "b2": b2}
        )
    return in_maps


def kernel(x, w, b):
    from concourse.bass_utils import run_bass_kernel_spmd

    if "nc" not in _CACHE:
        _CACHE["nc"] = _build_program()
    nc = _CACHE["nc"]

    in_maps = _prep_inputs(x, w, b)
    res = run_bass_kernel_spmd(nc, in_maps, list(range(N_CORES)))
    out = np.stack(
        [np.asarray(res.results[core]["out_t"], dtype=np.float32).T for core in range(N_CORES)]
    )
    return out.astype(np.float32)
